# revision 53
# baseline (speedup 1.0000x reference)
"""Trainium2 Bass kernel for nn_MultiHeadRecurrentActorNetwork (scatter_memory).

Math (per row b of B=131072):
  logits[0:2]   = f @ W_pick              (f = features[b], 256)
  logits[2:4]   = f @ W_partner
  logits[4:10]  = (f @ Wg_tw + bg_tw) @ E6^T,  E6 = card_table[CALL_IDS] @ We_tw + be_tw
  logits[106]   = f @ W_pu
  slot_scores[s] = v . tanh((f @ Wg_ptr) + tok[b,s] @ Wt_ptr)        s = 0..7
  card[c]  = slot_scores of the LAST slot s with hand_ids[b,s] == c, else NEG
  logits[10:42] = logits[42:74] = logits[74:106] = card[0:32]
  out = softmax(where(mask, logits, NEG))

Kernel strategy (8-way batch data parallelism, R = B/8 rows per core).
The machine balance is Act-bound: tanh runs only on the scalar engine at
1 elem/cycle/partition (65536 elems/partition -> ~55us pure), so every
other engine is kept strictly below that wall:

  * PE: token matmuls use fp8e4 DoubleRow (two 64-dim slot k-tiles packed
    along the free axis, 0.5 cy/col); the gptr broadcast-add into the token
    PSUM is a second DoubleRow matmul whose moving operand is the fp8 gpP
    tile with a 0-stride k-tile dim (reads g twice, I-weights route it to
    both parity halves).  gptr itself stays f16 (features are f16 for the
    precision-critical direct head).  ~30-44us.
  * Act: tanh in [128, 1536] instructions (3-bank PSUM uT tiles, x2
    ping-pong; 1-bank o64 + 1-bank scps = exactly 8 banks), exp batched
    over EXPG groups from the scps bank.
  * scatter/dedup: host precomputes scatter indices (dups -> -1,
    last-wins) and the 0/1 keep mask, packed into one int16 DMA; per-t
    local_scatter zero-fills its 32-col card block and writes normalized
    fp16 card probs DIRECTLY into the compact output tile.
  * output is compact: 44 f16 cols/row ([32 card | 11 direct | pad]; the
    card block leads so every local_scatter dst is 4-byte aligned --
    gpsimd scatter silently corrupts on 2-mod-4 byte offsets);
    the host replicates the card block to the 3 action ranges.  Output DMA
    drops 3.5MB -> 1.44MB/core.
  * DMA: ft f16 [256,R] / tok fp8 [64, 8, R] slot-major (64 partitions),
    4-group strips with per-group cuts during ramp-up, 4KB/2KB runs,
    ~52us total -- under the Act wall.  Token/index loads ride the gpsimd
    SWDGE queue so they parallelize with the SP feature loads.
"""

import numpy as np
import ml_dtypes

import concourse.bacc as bacc
import concourse.tile as tile
import concourse.mybir as mybir
from contextlib import ExitStack

F16 = mybir.dt.float16
F32 = mybir.dt.float32
F8 = mybir.dt.float8e4
I16 = mybir.dt.int16
OP = mybir.AluOpType
AF = mybir.ActivationFunctionType
AX = mybir.AxisListType
DR = mybir.MatmulPerfMode.DoubleRow

N_CORES = 8
A = 107
NEG = -1e8
CALL_CARD_IDS = np.array([0, 2, 4, 6, 8, 10])
F16H = np.float16
F8H = ml_dtypes.float8_e4m3fn

# pipeline tuning (module-level so the dev harness can sweep them)
TUNE = dict(depth=2, ftb=4, tkb=3, upool=7, lpool=4, epool=4, strip=4,
            tanh_ch=3, expg=4, normlag=9, gp_pool=0, gp_ahead=2,
            pdir_pool=1, ppub=2, pp64b=1, ppspb=1, tail_drain=1,
            store_dve=0, tok_dve=1, ramp_cut=4, gdirect=0)


# --------------------------------------------------------------------------
# device program
# --------------------------------------------------------------------------

def build_program(R, debug=False, reps=1):
    """One-core program processing R rows (R % 4096 == 0)."""
    assert R % 4096 == 0
    NG = R // 512          # groups of 512 rows (4 subtiles of 128 partitions)
    NT = R // 128          # 128-row subtiles

    nc = bacc.Bacc(None, target_bir_lowering=False, debug=debug)

    ft = nc.dram_tensor("ft", [256, R], F16, kind="ExternalInput").ap()
    tokt = nc.dram_tensor("tokt", [64, 8 * R], F8, kind="ExternalInput").ap()
    cpk = nc.dram_tensor("cpk", [128, 704], F16, kind="ExternalInput").ap()
    cpi = nc.dram_tensor("cpi", [128, 2 * NT * 8], I16,
                         kind="ExternalInput").ap()
    out = nc.dram_tensor("out", [128, NG * 176], F16, kind="ExternalOutput").ap()

    with tile.TileContext(nc) as tc, ExitStack() as ctx:
        if reps == 1:
            _body(ctx, tc, nc, NG, NT, ft, tokt, cpk, cpi, out)
        else:
            with tc.For_i(0, reps, 1):
                _body(ctx, tc, nc, NG, NT, ft, tokt, cpk, cpi, out)
    nc.compile()
    return nc


def _body(ctx, tc, nc, NG, NT, ft, tokt, cpk, cpi, out):
    TCH = TUNE["tanh_ch"]          # chunks (512 cols) per tanh instruction
    EXPG = TUNE["expg"]            # groups per scps bank / exp instruction
    DEPTH = TUNE["depth"]          # sc/dir emission lag (groups)

    assert NG % 4 == 0 and NG % EXPG == 0

    cpool = ctx.enter_context(tc.tile_pool(name="consts", bufs=1))
    ipool = ctx.enter_context(tc.tile_pool(name="ids", bufs=1))
    dpool = ctx.enter_context(tc.tile_pool(name="din", bufs=2))
    gpool = ctx.enter_context(tc.tile_pool(name="gp", bufs=3))
    upool = ctx.enter_context(tc.tile_pool(name="us", bufs=TUNE["upool"]))
    epool = ctx.enter_context(tc.tile_pool(name="es", bufs=TUNE["epool"]))
    s16p = ctx.enter_context(tc.tile_pool(name="s16", bufs=3))
    rpool = ctx.enter_context(tc.tile_pool(name="red", bufs=3))
    lpool = ctx.enter_context(tc.tile_pool(name="pout", bufs=TUNE["lpool"]))
    # PSUM: 2x TCH-bank uT + 1-bank o64 + scps (allocated in this order so
    # everything stays bank-aligned)
    ppu = ctx.enter_context(tc.tile_pool(name="pu", bufs=TUNE["ppub"], space="PSUM"))
    pp64 = ctx.enter_context(tc.tile_pool(name="p64", bufs=TUNE["pp64b"], space="PSUM"))
    ppsp = ctx.enter_context(tc.tile_pool(name="psp", bufs=TUNE["ppspb"], space="PSUM"))

    # ---- constants -------------------------------------------------------
    CPK = cpool.tile([128, 704], F16, tag="CPK")
    wg_t = [CPK[:, 64 * k:64 * k + 64] for k in range(2)]
    wdir_t = [CPK[:, 128 + 16 * k:128 + 16 * k + 16] for k in range(2)]
    # fp8 DoubleRow stationaries, [64, 2, 128] packed in f16 cols
    wt_dr = CPK[0:64, 160:288].bitcast(F8).rearrange("p (k f) -> p k f", k=2)
    sm_dr = CPK[0:64, 288:416].bitcast(F8).rearrange("p (k f) -> p k f", k=2)
    vmat_t = CPK[:, 416:448]
    # f16 duplicated-Wg stationaries for the ramp groups (g added into the
    # token PSUM directly, skipping the o64 -> gpP -> DR-smat chain)
    wgd_t = [CPK[:, 448 + 128 * k:448 + 128 * k + 128] for k in range(2)]
    CPI = ipool.tile([128, 2 * NT * 8], I16, tag="CPI")
    idxg = CPI[:, 0:NT * 8]
    keepf = CPI[:, NT * 8:2 * NT * 8].bitcast(F16)

    # ---- strip loads (plain contiguous DMA) ------------------------------
    MAXSTRIP = TUNE["strip"]
    W = 512 * MAXSTRIP

    def emit_ft(start_g, n, cuts=None):
        s0, rows = 512 * start_g, 512 * n
        FT = dpool.tile([128, 2 * W], F16, tag="FT", name="FT",
                        bufs=TUNE["ftb"])
        ft3 = FT[:].rearrange("p (k w) -> p k w", k=2)
        for a, b in zip(cuts or [0, rows], (cuts or [0, rows])[1:]):
            nc.sync.dma_start(
                ft3[:, :, a:b],
                ft[:, s0 + a:s0 + b].rearrange("(k p) c -> p k c", p=128))
        return ft3

    tok3 = tokt.rearrange("p (s r) -> p s r", s=8)

    tok_eng = nc.gpsimd if TUNE["tok_dve"] else nc.sync

    def emit_tok(start_g, n, cuts=None):
        s0, rows = 512 * start_g, 512 * n
        TK = dpool.tile([64, 8 * W], F8, tag="TK", name="TK",
                        bufs=TUNE["tkb"])
        tk3 = TK[:].rearrange("p (s w) -> p s w", s=8)
        for a, b in zip(cuts or [0, rows], (cuts or [0, rows])[1:]):
            tok_eng.dma_start(tk3[:, :, a:b], tok3[:, :, s0 + a:s0 + b])
        return tk3

    def emit_gptr(g, FT, qoff):
        """gptr head: o64 = Wg^T @ f -> [64, 512] psum -> fp8 SBUF."""
        o64 = pp64.tile([64, 512], F32, tag="o64")
        for k in range(2):
            q = slice(512 * qoff, 512 * qoff + 512)
            nc.tensor.matmul(o64[:], wg_t[k], FT[:, k, q],
                             start=(k == 0), stop=(k == 1))
        gpP = gpool.tile([64, 512], F8, tag="gpP")
        eng = nc.gpsimd if TUNE["gp_pool"] else nc.vector
        eng.tensor_copy(gpP[:], o64[:])
        return gpP[:]

    # global chunk bookkeeping: chunk C = 4*g + c; uT tile k = C // TCH
    NC = 4 * NG
    tile_start = {}
    for C in range(NC):
        k = C // TCH
        s = k * TCH
        tile_start[C] = (k, s, min(s + TCH, NC))
    cur = dict(uT=None, uS=None, k=-1)
    uS_tiles = {}

    FT_cur = [None, None]

    def emit_chunk(C, tk3, gpP, qoff):
        """token DoubleRow + g-broadcast DoubleRow into uT[k]; tanh when the
        tile fills."""
        k, s, e = tile_start[C]
        j = C - s
        if j == 0:
            cur["uT"] = ppu.tile([128, 512 * TCH], F32, tag="uT", name="uT")
            cur["k"] = k
        dst = cur["uT"][:, 512 * j:512 * j + 512]
        c = C % 4
        q = slice(512 * qoff, 512 * qoff + 512)
        nc.tensor.matmul(dst, wt_dr, tk3[:, 2 * c:2 * c + 2, q],
                         start=True, stop=False, perf_mode=DR)
        if gpP is None:
            # ramp path: accumulate g straight from the features (f16)
            for k in range(2):
                nc.tensor.matmul(dst, wgd_t[k], FT_cur[k][:, k, q],
                                 start=False, stop=(k == 1))
        else:
            gp2 = gpP.unsqueeze(1).broadcast_to([64, 2, 512])
            nc.tensor.matmul(dst, sm_dr, gp2, start=False, stop=True,
                             perf_mode=DR)
        if C == e - 1:
            w = 512 * (j + 1)
            uS = upool.tile([128, 512 * TCH], F16, tag="uS", name="uS")
            nc.scalar.activation(uS[:, 0:w], cur["uT"][:, 0:w], AF.Tanh)
            uS_tiles[cur["k"]] = uS

    SC = [None]

    def emit_scdir(g, FT, qoff, boff):
        """slot scores + direct logits for group g -> scps bank."""
        if boff == 0:
            # padded to a full 2KB bank so PSUM stays bank-aligned
            SC[0] = ppsp.tile([128, 512], F32, tag="scps", name="scps")
        scps = SC[0][:, 76 * boff:76 * boff + 76]
        for g2 in range(4):
            for c in range(4):
                Cc = 4 * g + c
                kk, ss, _ = tile_start[Cc]
                uS = uS_tiles[kk]
                off = 512 * (Cc - ss) + 128 * g2
                nc.tensor.matmul(scps[:, 19 * g2:19 * g2 + 8],
                                 uS[:, off:off + 128],
                                 vmat_t[:, 8 * c:8 * c + 8],
                                 start=(c == 0), stop=(c == 3))
            for k in range(2):
                sl = slice(512 * qoff + 128 * g2, 512 * qoff + 128 * g2 + 128)
                nc.tensor.matmul(scps[:, 19 * g2 + 8:19 * g2 + 19],
                                 FT[:, k, sl], wdir_t[k][:, 0:11],
                                 start=(k == 0), stop=(k == 1))
        return SC[0]

    def emit_exp(scps, nbg):
        """exp for a batch of nbg groups; frees the scps bank early."""
        es = epool.tile([128, 76 * EXPG], F32, tag="es", name="es")
        nc.scalar.activation(es[:, 0:76 * nbg], scps[:, 0:76 * nbg], AF.Exp)
        return es

    def emit_norm(g0, ng, es, coff):
        """normalize + scatter + store for groups [g0, g0+ng) -- emitted
        late enough that every dependency is already satisfied, so the
        in-order DVE / Pool / DMA queues never block on it."""
        nt = 4 * ng
        es3 = (es[:, coff:coff + 76 * ng]
               .rearrange("p (t e) -> p t e", e=19))      # [128, nt, 19]
        # denominator before the scatter (dup slots masked via keepf)
        es8k = rpool.tile([128, 64], F32, tag="es8k", name="es8k")[:, 0:8 * nt]
        keep3 = (keepf[:, 32 * g0:32 * (g0 + ng)]
                 .rearrange("p (t s) -> p t s", s=8))
        nc.vector.tensor_tensor(es8k.rearrange("p (t s) -> p t s", s=8),
                                es3[:, :, 0:8], keep3, OP.mult)
        denc = rpool.tile([128, 8], F32, tag="denc", name="denc")[:, 0:nt]
        nc.vector.tensor_reduce(denc, es8k.rearrange("p (t s) -> p t s", s=8),
                                AX.X, OP.add)
        dend = rpool.tile([128, 8], F32, tag="dend", name="dend")[:, 0:nt]
        nc.vector.tensor_reduce(dend, es3[:, :, 8:19], AX.X, OP.add)
        den = rpool.tile([128, 8], F32, tag="den", name="den")[:, 0:nt]
        nc.vector.scalar_tensor_tensor(den, denc, 3.0, dend,
                                       OP.mult, OP.add)
        rec = rpool.tile([128, 8], F32, tag="rec", name="rec")[:, 0:nt]
        nc.vector.reciprocal(rec, den)
        # normalized fp16 slot probs (contiguous scatter source)
        pn8 = s16p.tile([128, 64], F16, tag="pn8", name="pn8")[:, 0:8 * nt]
        rec8 = rec.unsqueeze(2).broadcast_to([128, nt, 8])
        nc.vector.tensor_tensor(pn8.rearrange("p (t s) -> p t s", s=8),
                                es3[:, :, 0:8], rec8, OP.mult)

        P = lpool.tile([128, 176 * ng], F16, tag="P", name="P")
        P3 = P[:].rearrange("p (t a) -> p t a", a=44)     # [128, nt, 44]
        # direct probs (cols 0:11 of each 44-col row block); card block and
        # direct cols are disjoint so this is independent of the scatters
        rec11 = rec.unsqueeze(2).broadcast_to([128, nt, 11])
        eng = nc.gpsimd if TUNE["pdir_pool"] else nc.vector
        eng.tensor_tensor(P3[:, :, 32:43], es3[:, :, 8:19], rec11, OP.mult)
        nc.vector.memset(P3[:, :, 43:44], 0)
        # per-t-block fp16 scatters zero-fill their 32-col card block and
        # place the card probs; empty cards come back exactly 0.0.  (the
        # card block leads each 44-col row block so every scatter dst is
        # 4-byte aligned -- gpsimd corrupts on 2-mod-4 byte offsets)
        for t in range(nt):
            nc.gpsimd.local_scatter(P[:, 44 * t:44 * t + 32],
                                    pn8[:, 8 * t:8 * t + 8],
                                    idxg[:, 32 * g0 + 8 * t:32 * g0 + 8 * t + 8],
                                    channels=128, num_elems=32, num_idxs=8)
        eng = nc.vector if TUNE["store_dve"] else nc.sync
        eng.dma_start(out[:, 176 * g0:176 * (g0 + ng)], P[:])

    # ---- software-pipelined emission ------------------------------------
    assert NG % 4 == 0
    if MAXSTRIP >= 4 and NG >= 12:
        # small leading strips smooth the DMA-bus ramp; small trailing
        # strips shorten the compute tail after the last load
        sizes = [2, 2] + [MAXSTRIP] * ((NG - 8) // MAXSTRIP) + [2, 1, 1]
    elif MAXSTRIP >= 2:
        sizes = [2] * ((NG - 2) // 2) + [1, 1]
    else:
        sizes = [1] * NG
    strips = []
    s0 = 0
    for n in sizes:
        strips.append((s0, n))
        s0 += n
    assert s0 == NG

    # group -> (strip index, strip-local offset)
    g2s = {}
    for si, (start, n) in enumerate(strips):
        for j in range(n):
            g2s[start + j] = (si, j)

    # exp batches: EXPG groups each; the last batch runs exp/normalize
    # PER GROUP inside its one live scps bank (no slot reuse -> no WAR),
    # so the final tail overlaps the last tanhs
    batches = [(g0, min(EXPG, NG - g0)) for g0 in range(0, NG, EXPG)]
    g2b = {}
    for bi, (bs, bn) in enumerate(batches):
        for k in range(bn):
            g2b[bs + k] = (bs, k, bn)

    ftq, tkq = {}, {}
    pend_sc = []      # (g, FT, qoff) awaiting sc/dir (+ exp per batch)
    pend_norm = []    # (unit, es, half) awaiting normalize/scatter/store
    NORMLAG = TUNE["normlag"]
    AHEAD = TUNE["gp_ahead"]
    gpq = {}

    # per-group DMA cuts for the first RAMPCUT strips: during ramp-up the
    # bus delivers group-by-group so the Act pipeline never waits on a
    # multi-group strip lump
    RAMPCUT = TUNE["ramp_cut"]

    def strip_cuts(si):
        n = strips[si][1]
        if si < RAMPCUT:
            return [512 * k for k in range(n + 1)]
        return None

    # wg weights first (gptr gate), then the first-group feature cut, then
    # the DoubleRow stationaries (chunk-0 gate) BEFORE the bulk strip loads
    nc.sync.dma_start(CPK[:, 0:128], cpk[:, 0:128])
    s0n = strips[0][1]
    FT0 = dpool.tile([128, 2 * W], F16, tag="FT", name="FT", bufs=TUNE["ftb"])
    ft30 = FT0[:].rearrange("p (k w) -> p k w", k=2)
    nc.sync.dma_start(ft30[:, :, 0:512],
                      ft[:, 0:512].rearrange("(k p) c -> p k c", p=128))
    nc.sync.dma_start(CPK[:, 128:448], cpk[:, 128:448])
    for a in range(512, 512 * s0n, 512):
        nc.sync.dma_start(
            ft30[:, :, a:a + 512],
            ft[:, a:a + 512].rearrange("(k p) c -> p k c", p=128))
    ftq[0] = ft30
    tkq[0] = emit_tok(*strips[0], cuts=strip_cuts(0))
    if len(strips) > 1:
        ftq[1] = emit_ft(*strips[1], cuts=strip_cuts(1))
        tkq[1] = emit_tok(*strips[1], cuts=strip_cuts(1))

    def emit_gptr_for(g):
        si, j = g2s[g]
        gpq[g] = emit_gptr(g, ftq[si], j)

    # prime AHEAD gptrs; the steady-state one is emitted AFTER each group's
    # chunk matmuls so the PE queue never parks chunk work behind a gptr
    # whose o64 slot is still owned by an unfinished gpP copy.  the first
    # GDIR groups use the ramp path (no gptr at all).
    GDIR = TUNE["gdirect"]
    for g in range(GDIR, min(GDIR + AHEAD, NG)):
        emit_gptr_for(g)

    def pop_sc():
        gb, FTb, qb = pend_sc.pop(0)
        bs, boff, bn = g2b[gb]
        scps = emit_scdir(gb, FTb, qb, boff)
        if bs + bn == NG:
            # final batch: per-group exp + 1-group norm units
            es = epool.tile([128, 76 * EXPG], F32, tag="es", name="es")
            nc.scalar.activation(es[:, 0:76], scps[:, 76 * boff:76 * boff + 76],
                                 AF.Exp)
            pend_norm.append((gb, 1, es, 0))
        elif boff == bn - 1:
            es = emit_exp(scps, bn)
            k = 0
            while k < bn:
                ng = min(2, bn - k)
                pend_norm.append((bs + k, ng, es, 76 * k))
                k += ng

    def pop_norm():
        emit_norm(*pend_norm.pop(0))

    for si, (start, n) in enumerate(strips):
        FT = ftq[si]
        TK = tkq.pop(si)
        for j in range(n):
            g = start + j
            if j == 0 and si + 2 < len(strips):
                tkq[si + 2] = emit_tok(*strips[si + 2], cuts=strip_cuts(si + 2))
                ftq[si + 2] = emit_ft(*strips[si + 2], cuts=strip_cuts(si + 2))
            if g == 2:
                tok_eng.dma_start(CPI[:], cpi[:])
            gp_g = gpq.pop(g) if g >= GDIR else None
            FT_cur[0] = FT_cur[1] = FT
            for c in range(4):
                emit_chunk(4 * g + c, TK, gp_g, j)
            if GDIR <= g + AHEAD < NG and g + AHEAD not in gpq:
                emit_gptr_for(g + AHEAD)
            pend_sc.append((g, FT, j))
            if len(pend_sc) > DEPTH:
                pop_sc()
            while pend_norm and pend_norm[0][0] + NORMLAG <= g:
                pop_norm()
            if TUNE["tail_drain"] and g >= NG - 4:
                if pend_sc:
                    pop_sc()
                if pend_norm:
                    pop_norm()
    while pend_sc:
        pop_sc()
    while pend_norm:
        pop_norm()


# --------------------------------------------------------------------------
# host side
# --------------------------------------------------------------------------

_PROGRAMS = {}


def _get_program(R):
    if R not in _PROGRAMS:
        _PROGRAMS[R] = build_program(R)
    return _PROGRAMS[R]


def _pack_f8(w):
    """[p, n] fp8 array -> [p, n//2] f16-bitcast view for cpk packing."""
    u8 = np.ascontiguousarray(w).view(np.uint8)
    packed = (u8[:, 0::2].astype(np.uint16)
              | (u8[:, 1::2].astype(np.uint16) << 8))
    return packed.view(F16H)


def _prep_weights(i):
    f32 = lambda x: np.asarray(x, np.float32)
    ct = f32(i["card_table"])
    E6 = ct[CALL_CARD_IDS] @ f32(i["We_tw"]) + f32(i["be_tw"])      # (6, 64)
    Wcall = f32(i["Wg_tw"]) @ E6.T                                   # (256, 6)
    bcall = E6 @ f32(i["bg_tw"])                                     # (6,)
    Wdir = np.concatenate([f32(i["W_pick"]), f32(i["W_partner"]),
                           Wcall, f32(i["W_pu"])], axis=1)           # (256, 11)
    bdir = np.concatenate([f32(i["b_pick"]), f32(i["b_partner"]),
                           bcall, f32(i["b_pu"])])
    bptr = f32(i["bg_ptr"]) + f32(i["bt_ptr"])
    wdir16 = np.zeros((256, 16), F16H)
    wdir16[:, 0:11] = Wdir.astype(F16H)
    wg16 = f32(i["Wg_ptr"]).astype(F16H)                             # (256, 64)
    wt8 = f32(i["Wt_ptr"]).astype(np.float32).astype(F8H)            # (64, 64)
    # token DoubleRow stationary [64, 2, 128]: k-tile 0 = slot 2c -> cols
    # 0:64, k-tile 1 = slot 2c+1 -> cols 64:128
    wt_dr = np.zeros((64, 2, 128), F8H)
    wt_dr[:, 0, 0:64] = wt8
    wt_dr[:, 1, 64:128] = wt8
    # g-broadcast stationary: k-tile 0 routes g (identity) to BOTH parity
    # halves; k-tile 1 (same g data via 0-stride) contributes nothing
    sm_dr = np.zeros((64, 2, 128), F8H)
    eye = np.eye(64, dtype=F8H)
    sm_dr[:, 0, 0:64] = eye
    sm_dr[:, 0, 64:128] = eye
    v = f32(i["v_ptr"])
    vmat = np.zeros((128, 32), F16H)
    for c in range(4):
        for sp in range(2):
            vmat[sp * 64:(sp + 1) * 64, 8 * c + 2 * c + sp] = v.astype(F16H)
    cpk = np.zeros((128, 704), F16H)
    cpk[:, 0:64] = wg16[0:128]
    cpk[:, 64:128] = wg16[128:256]
    cpk[:, 128:144] = wdir16[0:128]
    cpk[:, 144:160] = wdir16[128:256]
    cpk[0:64, 160:288] = _pack_f8(wt_dr.reshape(64, 256))
    cpk[0:64, 288:416] = _pack_f8(sm_dr.reshape(64, 256))
    cpk[:, 416:448] = vmat
    cpk[:, 448:512] = wg16[0:128]
    cpk[:, 512:576] = wg16[0:128]
    cpk[:, 576:640] = wg16[128:256]
    cpk[:, 640:704] = wg16[128:256]
    return dict(cpk=cpk), bdir, bptr


def _core_inputs(weights, f, tok, ids, r_lo, r_hi):
    R = r_hi - r_lo
    NT = R // 128
    ftc = np.ascontiguousarray(f[r_lo:r_hi].T, dtype=F16H)            # (256, R)
    # tokens slot-major on 64 partitions: tokt[d, s*R + r] = tok[r, s, d]
    tokc = np.ascontiguousarray(
        tok[r_lo:r_hi].transpose(2, 1, 0).reshape(64, 8 * R), dtype=F8H)
    ids8 = ids[r_lo:r_hi].astype(np.int64)                            # (R, 8)
    # keep = last occurrence of each card id in the row
    dup = np.zeros((R, 8), bool)
    for d in range(1, 8):
        dup[:, :8 - d] |= ids8[:, :8 - d] == ids8[:, d:]
    keep = ~dup
    idx = np.where(keep, ids8, -1).astype(np.int16)
    perm = lambda x: np.ascontiguousarray(
        x.reshape(NT, 128, 8).transpose(1, 0, 2).reshape(128, NT * 8))
    idxc = perm(idx)
    keepc = perm(keep.astype(F16H)).view(np.int16)
    cpi = np.concatenate([idxc, keepc], axis=1)
    return dict(ft=ftc, tokt=tokc, cpk=weights["cpk"],
                cpi=np.ascontiguousarray(cpi))


def _unshard_out(o, R):
    """[128, NG*176] fp16 device layout -> [R, 107] f32."""
    NG = R // 512
    c = (np.asarray(o).reshape(128, NG, 4, 44)
         .transpose(1, 2, 0, 3).reshape(R, 44).astype(np.float32))
    full = np.empty((R, A), np.float32)
    full[:, 0:10] = c[:, 32:42]
    full[:, 106] = c[:, 42]
    card = c[:, 0:32]
    full[:, 10:42] = card
    full[:, 42:74] = card
    full[:, 74:106] = card
    return full


def _reference_numpy(i):
    """Plain numpy replica of reference.py (fallback for unexpected inputs)."""
    f = np.asarray(i["features"], np.float32)
    tok = np.asarray(i["hand_tokens"], np.float32)
    ids = np.asarray(i["hand_ids"], np.int64)
    mask = np.asarray(i["action_mask"], bool)
    B = f.shape[0]
    logits = np.full((B, A), NEG, np.float32)
    logits[:, 0:2] = f @ np.asarray(i["W_pick"], np.float32) + np.asarray(i["b_pick"], np.float32)
    partner = f @ np.asarray(i["W_partner"], np.float32) + np.asarray(i["b_partner"], np.float32)
    logits[:, 2] = partner[:, 0]
    logits[:, 3] = partner[:, 1]
    E = np.asarray(i["card_table"], np.float32) @ np.asarray(i["We_tw"], np.float32) + np.asarray(i["be_tw"], np.float32)
    S = (f @ np.asarray(i["Wg_tw"], np.float32) + np.asarray(i["bg_tw"], np.float32)) @ E.T
    logits[:, 4:10] = S[:, CALL_CARD_IDS]
    e = np.tanh((f @ np.asarray(i["Wg_ptr"], np.float32) + np.asarray(i["bg_ptr"], np.float32))[:, None, :]
                + tok @ np.asarray(i["Wt_ptr"], np.float32) + np.asarray(i["bt_ptr"], np.float32))
    slot_scores = e @ np.asarray(i["v_ptr"], np.float32)
    rows = np.arange(B)
    for base in (10, 42, 74):
        for s in range(8):
            cid = ids[:, s]
            ok = cid < 32
            logits[rows[ok], base + cid[ok]] = slot_scores[ok, s]
    logits[:, 106] = (f @ np.asarray(i["W_pu"], np.float32) + np.asarray(i["b_pu"], np.float32))[:, 0]
    logits = np.where(mask, logits, NEG)
    x = logits - logits.max(axis=1, keepdims=True)
    ex = np.exp(x)
    return ex / ex.sum(axis=1, keepdims=True)


def kernel(**inputs):
    from concourse.bass_utils import run_bass_kernel_spmd

    f = np.asarray(inputs["features"], np.float32)
    tok = np.asarray(inputs["hand_tokens"], np.float32)
    ids = np.asarray(inputs["hand_ids"])
    mask = np.asarray(inputs["action_mask"], bool)
    B = f.shape[0]

    weights, bdir, bptr = _prep_weights(inputs)
    irregular = (B % (N_CORES * 4096) != 0 or not mask.all()
                 or np.any(bdir != 0) or np.any(bptr != 0)
                 or ids.min() < 0 or ids.max() >= 32)
    if irregular:
        return _reference_numpy(inputs)

    R = B // N_CORES
    nc = _get_program(R)
    in_maps = [_core_inputs(weights, f, tok, ids, i * R, (i + 1) * R)
               for i in range(N_CORES)]
    res = run_bass_kernel_spmd(nc, in_maps, list(range(N_CORES)))
    return np.concatenate([_unshard_out(res.results[i]["out"], R)
                           for i in range(N_CORES)], axis=0)
